# revision 22
# baseline (speedup 1.0000x reference)
"""Trainium2 Bass kernel for Nucleus1 spatial axial attention.

Reference computation (per frame of a B=1,T=16,H=64,W=64,C=768 video):
  qkv = x @ w_in + b_in ; per-head LN(q), LN(k) (head_dim=96); axial RoPE on
  first 64 dims of each head; softmax attention independently along W (rows)
  and along H (cols); out = 0.5*(xx+xy) @ w_out + b_out.

Sharding: data-parallel over T. Each of the 8 NeuronCores processes 2 frames
end to end; no collectives. Inside a core everything is computed per frame,
per head, with feature-major q/k (via HWDGE DMA transpose), token-major v
blocks, and PE matmuls for all projections and attention.

Host-side preprocessing folds gamma, the 1/sqrt(96) attention scale and the
0.5 merge factor into tables/weights; RoPE pair-interleaving is removed by
permuting w_in columns (attention dot products are permutation invariant).
"""

import math
import sys

import numpy as np

sys.path.insert(0, "/opt/trn_rl_repo")

import ml_dtypes

import concourse.bass as bass
import concourse.bacc as bacc
import concourse.mybir as mybir
from concourse import tile

BF16 = ml_dtypes.bfloat16
F32 = np.float32

EMBED = 768
NHEADS = 8
HD = 96          # head dim
ROT = 64         # rotary dims per head
NFREQ = 32       # 16 (h-axis) + 16 (w-axis) frequency slots
SEQ = 64
TOK = SEQ * SEQ  # tokens per frame
KC = EMBED // 128  # 6 contraction chunks
NCORES = 8
EPS = 1e-5

dt = mybir.dt
ALU = mybir.AluOpType
ACTF = mybir.ActivationFunctionType
AX = mybir.AxisListType


# ---------------------------------------------------------------- host prep

def _perm96() -> np.ndarray:
    """Internal head-dim order: rope evens, rope odds, pass-through."""
    return np.array(
        list(range(0, ROT, 2)) + list(range(1, ROT, 2)) + list(range(ROT, HD)),
        dtype=np.int64,
    )


def _freqs() -> np.ndarray:
    """[TOK, 32] un-repeated axial frequencies: concat(fh[y], fw[x])."""
    base = np.linspace(1.0, 128.0, 16, dtype=np.float64) * np.pi
    pos = np.linspace(-1.0, 1.0, SEQ, dtype=np.float64)
    f = pos[:, None] * base[None, :]                      # [64, 16]
    fh = np.broadcast_to(f[:, None, :], (SEQ, SEQ, 16))   # per y
    fw = np.broadcast_to(f[None, :, :], (SEQ, SEQ, 16))   # per x
    return np.concatenate([fh, fw], -1).reshape(TOK, NFREQ).astype(np.float64)


def _tables(q_gamma, q_beta, k_gamma, k_beta):
    """A, B, C tables [TOK, 2, 96] fp32 in internal (permuted) head-dim order.

    q_final' = xhat'*A + swap32(xhat')*B + C   (xhat = LN-normalized, no affine)
    where swap32 swaps the two 32-blocks of the rotary part. The q tables
    (j=0) additionally fold the 1/sqrt(96) attention scale.
    """
    P = _perm96()
    F = _freqs()
    cos = np.cos(F)
    sin = np.sin(F)
    A = np.zeros((TOK, 2, HD), np.float64)
    B = np.zeros((TOK, 2, HD), np.float64)
    C = np.zeros((TOK, 2, HD), np.float64)
    for j, (g, b) in enumerate([(q_gamma, q_beta), (k_gamma, k_beta)]):
        gp = np.asarray(g, np.float64)[P]
        bp = np.asarray(b, np.float64)[P]
        sc = 1.0 / math.sqrt(HD) if j == 0 else 1.0
        A[:, j, 0:32] = cos * gp[0:32] * sc
        A[:, j, 32:64] = cos * gp[32:64] * sc
        A[:, j, 64:96] = gp[64:96] * sc
        B[:, j, 0:32] = -sin * gp[32:64] * sc
        B[:, j, 32:64] = sin * gp[0:32] * sc
        C[:, j, 0:32] = (bp[0:32] * cos - bp[32:64] * sin) * sc
        C[:, j, 32:64] = (bp[32:64] * cos + bp[0:32] * sin) * sc
        C[:, j, 64:96] = bp[64:96] * sc
    return A.astype(F32), B.astype(F32), C.astype(F32)


def _prep_weights(w_in, w_out):
    """Permuted/padded weights, all bf16.

    Returns wqk [768, NH*192] (per head: q 96 perm'd | k 96 perm'd),
    wv [768, NH*128] (head-dim padded 96->128 with zeros),
    wo [NH*128, 768] (rows padded; 0.5 merge factor folded in).
    """
    P = _perm96()
    w = np.asarray(w_in).astype(F32)
    wq = w[:, 0:768].reshape(EMBED, NHEADS, HD)[:, :, P]
    wk = w[:, 768:1536].reshape(EMBED, NHEADS, HD)[:, :, P]
    wqk = np.stack([wq, wk], axis=2).reshape(EMBED, NHEADS * 2 * HD)
    wv = w[:, 1536:2304].reshape(EMBED, NHEADS, HD)
    wvp = np.zeros((EMBED, NHEADS, 128), F32)
    wvp[:, :, 0:HD] = wv
    wo = np.asarray(w_out).astype(F32)
    wop = np.zeros((NHEADS, 128, EMBED), F32)
    wop[:, 0:HD, :] = wo.reshape(NHEADS, HD, EMBED) * 0.5
    return (
        wqk.astype(BF16),
        wvp.reshape(EMBED, NHEADS * 128).astype(BF16),
        wop.reshape(NHEADS * 128, EMBED).astype(BF16),
    )


# ------------------------------------------------------------- bass program

def _emit(nc, tc, ctx, nframes, nheads, use_bin, use_beta, use_fbias, debug=False):
    x_d = nc.dram_tensor("x", [nframes, TOK, EMBED], dt.float32, kind="ExternalInput")
    wqk_d = nc.dram_tensor("wqk", [EMBED, nheads * 192], dt.bfloat16, kind="ExternalInput")
    wv_d = nc.dram_tensor("wv", [EMBED, nheads * 128], dt.bfloat16, kind="ExternalInput")
    wo_d = nc.dram_tensor("wo", [nheads * 128, EMBED], dt.bfloat16, kind="ExternalInput")
    tabA_d = nc.dram_tensor("tabA", [TOK, 2 * ROT], dt.bfloat16, kind="ExternalInput")
    tabB_d = nc.dram_tensor("tabB", [TOK, 2 * ROT], dt.bfloat16, kind="ExternalInput")
    gpass_d = nc.dram_tensor("gpass", [128, 2 * 32], dt.bfloat16, kind="ExternalInput")
    if use_beta:
        tabC_d = nc.dram_tensor("tabC", [TOK, 2 * HD], dt.bfloat16, kind="ExternalInput")
    if use_bin:
        bqk_d = nc.dram_tensor("bqk", [1, nheads * 192], dt.float32, kind="ExternalInput")
    if use_fbias:
        fb_d = nc.dram_tensor("fbias", [128, EMBED], dt.float32, kind="ExternalInput")
    out_d = nc.dram_tensor("out", [nframes, TOK, EMBED], dt.float32, kind="ExternalOutput")
    if debug:
        dbg_xT = nc.dram_tensor("dbg_xT", [KC, 128, TOK], dt.bfloat16, kind="ExternalOutput")
        dbg_qkt = nc.dram_tensor("dbg_qkt", [128, 32 * 256], dt.bfloat16, kind="ExternalOutput")
        dbg_qT = nc.dram_tensor("dbg_qT", [128, TOK], dt.bfloat16, kind="ExternalOutput")
        dbg_kT = nc.dram_tensor("dbg_kT", [128, TOK], dt.bfloat16, kind="ExternalOutput")
        dbg_vT = nc.dram_tensor("dbg_vT", [128, TOK], dt.bfloat16, kind="ExternalOutput")
        dbg_OT = nc.dram_tensor("dbg_OT", [128, TOK], dt.bfloat16, kind="ExternalOutput")

    cpool = ctx.enter_context(tc.sbuf_pool(name="cpool", bufs=1))
    xpool = ctx.enter_context(tc.sbuf_pool(name="xpool", bufs=KC))
    wpool = ctx.enter_context(tc.sbuf_pool(name="wpool", bufs=2))
    qkpool = ctx.enter_context(tc.sbuf_pool(name="qkpool", bufs=1))
    scrpool = ctx.enter_context(tc.sbuf_pool(name="scrpool", bufs=1))
    stpool = ctx.enter_context(tc.sbuf_pool(name="stpool", bufs=1))
    tpool = ctx.enter_context(tc.sbuf_pool(name="tpool", bufs=2))
    vpool = ctx.enter_context(tc.sbuf_pool(name="vpool", bufs=1))
    opool = ctx.enter_context(tc.sbuf_pool(name="opool", bufs=nheads))
    apool = ctx.enter_context(tc.sbuf_pool(name="apool", bufs=2))
    vbpool = ctx.enter_context(tc.sbuf_pool(name="vbpool", bufs=4))
    outpool = ctx.enter_context(tc.sbuf_pool(name="outpool", bufs=1))
    wopool = ctx.enter_context(tc.sbuf_pool(name="wopool", bufs=1))
    ps1 = ctx.enter_context(tc.psum_pool(name="ps1", bufs=2))

    # constants: rope tables (resident), identity for PE transpose
    tabA = cpool.tile([128, 32, 2 * ROT], dt.bfloat16)
    tabB = cpool.tile([128, 32, 2 * ROT], dt.bfloat16)
    gpass = cpool.tile([128, 2, 32], dt.bfloat16)
    nc.sync.dma_start(out=tabA, in_=tabA_d[:].rearrange("(m p) c -> p m c", p=128))
    nc.sync.dma_start(out=tabB, in_=tabB_d[:].rearrange("(m p) c -> p m c", p=128))
    nc.sync.dma_start(out=gpass, in_=gpass_d[:].rearrange("p (j c) -> p j c", c=32))
    if use_beta:
        tabC = cpool.tile([128, 32, 2 * HD], dt.bfloat16)
        nc.sync.dma_start(out=tabC, in_=tabC_d[:].rearrange("(m p) c -> p m c", p=128))
    if use_bin:
        bqk = cpool.tile([1, nheads * 192], dt.float32)
        nc.sync.dma_start(out=bqk, in_=bqk_d[:])
        ones1 = cpool.tile([1, 128], dt.bfloat16)
        nc.vector.memset(ones1, 1.0)
    if use_fbias:
        fbias = cpool.tile([128, EMBED], dt.float32)
        nc.sync.dma_start(out=fbias, in_=fb_d[:])
    ident_d = nc.dram_tensor("ident", [128, 128], dt.bfloat16, kind="ExternalInput")
    ident = cpool.tile([128, 128], dt.bfloat16)
    nc.sync.dma_start(out=ident, in_=ident_d[:])

    for f in range(nframes):
        # ---- transposed+cast load: xT[kc] [128c, 4096t] bf16
        xT = []
        for kc in range(KC):
            xt = xpool.tile([128, TOK], dt.bfloat16, name=f"xT{kc}", tag="xT",
                            bufs=KC)
            xT.append(xt)
        for m in range(32):
            xcast = xpool.tile([128, EMBED], dt.bfloat16, tag="xcast", bufs=2,
                               name="xcast")
            nc.gpsimd.dma_start(out=xcast, in_=x_d[f][m * 128:(m + 1) * 128, :])
            for kc in range(KC):
                nc.sync.dma_start(out=xT[kc][:, m * 128:(m + 1) * 128],
                                  in_=xcast[:, kc * 128:(kc + 1) * 128],
                                  transpose=True)

        if debug and f == 0:
            for kc in range(KC):
                nc.sync.dma_start(out=dbg_xT[kc], in_=xT[kc])
        OTs = []
        for h in range(nheads):
            # ---- per-head weights
            wqk_h = wpool.tile([128, KC, 192], dt.bfloat16, tag="wqk")
            nc.sync.dma_start(
                out=wqk_h,
                in_=wqk_d[:, h * 192:(h + 1) * 192].rearrange("(k p) c -> p k c", p=128),
            )
            wv_h = wpool.tile([128, KC, 128], dt.bfloat16, tag="wv")
            nc.sync.dma_start(
                out=wv_h,
                in_=wv_d[:, h * 128:(h + 1) * 128].rearrange("(k p) c -> p k c", p=128),
            )

            # ---- v projection (feature-major, head-dim padded)
            vT_h = vpool.tile([128, TOK], dt.bfloat16, tag="vT")
            for n in range(TOK // 512):
                pv = ps1.tile([128, 512], dt.float32, tag="psp")
                for kc in range(KC):
                    nc.tensor.matmul(
                        pv, wv_h[:, kc, :], xT[kc][:, n * 512:(n + 1) * 512],
                        start=(kc == 0), stop=(kc == KC - 1),
                    )
                nc.scalar.copy(vT_h[:, n * 512:(n + 1) * 512], pv)

            # ---- qk projection (token-major, padded slots for DMA transpose)
            qkt = qkpool.tile([128, 32, 2, 128], dt.bfloat16, tag="qkt")
            nc.vector.memset(qkt[:, :, :, HD:128], 0.0)
            for m in range(32):
                pqt = ps1.tile([128, 512], dt.float32, tag="psp", name="pqt")
                pq = pqt[:, 0:192]
                for kc in range(KC):
                    nc.tensor.matmul(
                        pq, xT[kc][:, m * 128:(m + 1) * 128], wqk_h[:, kc, :],
                        start=(kc == 0), stop=(kc == KC - 1 and not use_bin),
                    )
                if use_bin:
                    nc.tensor.matmul(
                        pq, ones1, bqk[:, h * 192:(h + 1) * 192],
                        start=False, stop=True,
                    )
                nc.scalar.copy(
                    qkt[:, m, :, 0:HD],
                    pq.rearrange("p (j c) -> p j c", c=HD),
                )

            # ---- layernorm stats over head dim (bn_stats: even/odd halves)
            X = qkt[:, :, :, 0:HD]
            sums = stpool.tile([128, 32, 2], dt.float32, tag="sums", name="sums")
            sumsq = stpool.tile([128, 32, 2], dt.float32, tag="sumsq", name="sumsq")
            nc.vector.tensor_reduce(sums, X, axis=AX.X, op=ALU.add)
            for qtr in range(4):
                sl = slice(8 * qtr, 8 * qtr + 8)
                sqs = scrpool.tile([128, 8, 2, HD], dt.bfloat16, tag="scr",
                                   name="sqs")
                nc.vector.tensor_mul(sqs, X[:, sl], X[:, sl])
                nc.vector.tensor_reduce(sumsq[:, sl], sqs, axis=AX.X, op=ALU.add)
            mu = stpool.tile([128, 32, 2], dt.float32, tag="mu")
            t0 = stpool.tile([128, 32, 2], dt.float32, tag="t0")
            rstd = stpool.tile([128, 32, 2], dt.float32, tag="rstd")
            nc.vector.tensor_scalar(mu, sums, 1.0 / HD, None, op0=ALU.mult)
            nc.vector.tensor_mul(t0, mu, mu)
            nc.vector.scalar_tensor_tensor(t0, sumsq, 1.0 / HD, t0,
                                           op0=ALU.mult, op1=ALU.subtract)
            nc.vector.tensor_scalar(t0, t0, EPS, None, op0=ALU.add)
            nc.vector.reciprocal(t0, t0)
            nc.scalar.sqrt(rstd, t0)

            # ---- normalize + rope (in place on X)
            mu_b = mu[:, :, :, None].broadcast_to((128, 32, 2, HD))
            rstd_b = rstd[:, :, :, None].broadcast_to((128, 32, 2, HD))
            nc.vector.tensor_sub(X, X, mu_b)
            nc.vector.tensor_mul(X, X, rstd_b)
            tabBv = tabB.rearrange("p m (j c) -> p m j c", c=ROT)
            tabAv = tabA.rearrange("p m (j c) -> p m j c", c=ROT)
            gp_b = gpass[:, None, :, :].broadcast_to((128, 32, 2, 32))
            nc.vector.tensor_mul(X[:, :, :, ROT:HD], X[:, :, :, ROT:HD], gp_b)
            for u in range(2):
                sl = slice(16 * u, 16 * u + 16)
                scr = scrpool.tile([128, 16, 2, ROT], dt.bfloat16, tag="scr",
                                   name="scr")
                nc.vector.tensor_mul(scr[:, :, :, 0:32], X[:, sl, :, 32:64],
                                     tabBv[:, sl, :, 0:32])
                nc.vector.tensor_mul(scr[:, :, :, 32:64], X[:, sl, :, 0:32],
                                     tabBv[:, sl, :, 32:64])
                nc.vector.tensor_mul(X[:, sl, :, 0:ROT], X[:, sl, :, 0:ROT],
                                     tabAv[:, sl])
                nc.vector.tensor_add(X[:, sl, :, 0:ROT], X[:, sl, :, 0:ROT], scr)
            if use_beta:
                tabCv = tabC.rearrange("p m (j c) -> p m j c", c=HD)
                nc.vector.tensor_add(X, X, tabCv)

            # ---- feature-major q/k via DMA transpose of 128x128 slots
            if debug and f == 0 and h == 0:
                nc.sync.dma_start(out=dbg_qkt[:], in_=qkt[:].rearrange("p a b c -> p (a b c)"))
            qT_h = tpool.tile([128, TOK], dt.bfloat16, tag="qT")
            kT_h = tpool.tile([128, TOK], dt.bfloat16, tag="qT")
            for m in range(32):
                nc.sync.dma_start(out=qT_h[:, m * 128:(m + 1) * 128],
                                  in_=qkt[:, m, 0, :], transpose=True)
                nc.sync.dma_start(out=kT_h[:, m * 128:(m + 1) * 128],
                                  in_=qkt[:, m, 1, :], transpose=True)

            # ---- axial attention, both directions
            if debug and f == 0 and h == 0:
                nc.sync.dma_start(out=dbg_qT[:], in_=qT_h)
                nc.sync.dma_start(out=dbg_kT[:], in_=kT_h)
                nc.sync.dma_start(out=dbg_vT[:], in_=vT_h)
            OT_h = opool.tile([128, TOK], dt.bfloat16, name=f"OT{h}", tag="OT")
            OTs.append(OT_h)
            qT_x = qT_h.rearrange("p (y x) -> p x y", x=SEQ)
            kT_x = kT_h.rearrange("p (y x) -> p x y", x=SEQ)
            vT_x = vT_h.rearrange("p (y x) -> p x y", x=SEQ)
            OT_x = OT_h.rearrange("p (y x) -> p x y", x=SEQ)
            for d in range(2):
                for bg in range(8):
                    pS = ps1.tile([128, 512], dt.float32, tag="psS")
                    for i in range(4):
                        b = bg * 4 + i
                        if d == 0:
                            nc.tensor.matmul(pS[:, i * 128:(i + 1) * 128],
                                             qT_h[0:HD, b * 128:(b + 1) * 128],
                                             kT_h[0:HD, b * 128:(b + 1) * 128],
                                             start=True, stop=True)
                        else:
                            for c2 in range(2):
                                o = i * 128 + c2 * 64
                                nc.tensor.matmul(
                                    pS[c2 * 64:c2 * 64 + 64, o:o + 64],
                                    qT_x[0:HD, 2 * b + c2, :],
                                    kT_x[0:HD, 2 * b + c2, :],
                                    start=True, stop=True)
                    P_t = apool.tile([128, 512], dt.bfloat16, tag="P")
                    if d == 0:
                        nc.scalar.activation(P_t, pS, ACTF.Exp)
                    else:
                        Sv = pS.rearrange("p (i c) -> p i c", c=128)
                        Ev = P_t.rearrange("p (i c) -> p i c", c=128)
                        nc.scalar.activation(Ev[0:SEQ, :, 0:SEQ],
                                             Sv[0:SEQ, :, 0:SEQ], ACTF.Exp)
                        nc.scalar.activation(Ev[SEQ:128, :, SEQ:128],
                                             Sv[SEQ:128, :, SEQ:128], ACTF.Exp)
                    Pv = P_t.rearrange("p (i c) -> p i c", c=128)
                    nc.vector.memset(Pv[0:SEQ, :, SEQ:128], 0.0)
                    nc.vector.memset(Pv[SEQ:128, :, 0:SEQ], 0.0)
                    ssum = stpool.tile([128, 4], dt.float32, tag="ssum")
                    nc.vector.tensor_reduce(ssum, Pv, axis=AX.X, op=ALU.add)
                    nc.vector.reciprocal(ssum, ssum)
                    nc.vector.tensor_mul(
                        P_t, P_t, ssum[:, :, None].broadcast_to((128, 4, 128)))
                    PT_t = apool.tile([128, 512], dt.bfloat16, tag="PT")
                    for i in range(4):
                        nc.sync.dma_start(out=PT_t[:, i * 128:(i + 1) * 128],
                                          in_=P_t[:, i * 128:(i + 1) * 128],
                                          transpose=True)
                    pO = ps1.tile([128, 512], dt.float32, tag="psO", bufs=3)
                    for i in range(4):
                        b = bg * 4 + i
                        vblk = vbpool.tile([128, 128], dt.bfloat16, tag="vb")
                        if d == 0:
                            nc.sync.dma_start(
                                out=vblk, in_=vT_h[:, b * 128:(b + 1) * 128],
                                transpose=True)
                        else:
                            pvb = ps1.tile([128, 128], dt.bfloat16, tag="pvb", bufs=1, name="pvb")
                            for c2 in range(2):
                                nc.tensor.transpose(pvb[c2 * 64:c2 * 64 + 64, :],
                                                    vT_x[:, 2 * b + c2, :], ident)
                            nc.scalar.copy(vblk, pvb)
                        nc.tensor.matmul(pO[:, i * 128:(i + 1) * 128], vblk,
                                         PT_t[:, i * 128:(i + 1) * 128],
                                         start=True, stop=True)
                    if d == 0:
                        nc.scalar.copy(OT_h[:, bg * 512:(bg + 1) * 512], pO)
                    else:
                        dst = OT_x[:, 8 * bg:8 * bg + 8, :]
                        nc.vector.tensor_add(
                            dst, dst, pO.rearrange("p (x y) -> p x y", y=SEQ))

        if debug and f == 0:
            nc.sync.dma_start(out=dbg_OT[:], in_=OTs[0])
        # ---- output projection
        wo_t = wopool.tile([128, nheads, EMBED], dt.bfloat16, tag="wo")
        nc.sync.dma_start(out=wo_t, in_=wo_d[:].rearrange("(h p) c -> p h c", p=128))
        for m in range(32):
            po1 = ps1.tile([128, 512], dt.float32, tag="psS", name="po1")
            po2 = ps1.tile([128, 512], dt.float32, tag="psS", name="po2")
            for h in range(nheads):
                nc.tensor.matmul(po1[:, 0:384], OTs[h][:, m * 128:(m + 1) * 128],
                                 wo_t[:, h, 0:384],
                                 start=(h == 0), stop=(h == nheads - 1))
                nc.tensor.matmul(po2[:, 0:384], OTs[h][:, m * 128:(m + 1) * 128],
                                 wo_t[:, h, 384:768],
                                 start=(h == 0), stop=(h == nheads - 1))
            osb = outpool.tile([128, EMBED], dt.float32, tag="osb")
            if use_fbias:
                nc.vector.tensor_add(osb[:, 0:384], po1[:, 0:384], fbias[:, 0:384])
                nc.vector.tensor_add(osb[:, 384:768], po2[:, 0:384], fbias[:, 384:768])
            else:
                nc.scalar.copy(osb[:, 0:384], po1[:, 0:384])
                nc.scalar.copy(osb[:, 384:768], po2[:, 0:384])
            nc.sync.dma_start(out=out_d[f, m * 128:(m + 1) * 128, :], in_=osb)


def build_program(nframes=2, nheads=NHEADS, use_bin=False, use_beta=False,
                  use_fbias=False, debug=False):
    from contextlib import ExitStack

    nc = bacc.Bacc()
    with ExitStack() as ctx:
        tc = ctx.enter_context(tile.TileContext(nc))
        _emit(nc, tc, ctx, nframes, nheads, use_bin, use_beta, use_fbias, debug=debug)
    nc.finalize()
    return nc


# ------------------------------------------------------------------- driver

_CACHE = {}
LAST_RESULT = None


def _inputs_for_core(xs, wqk, wv, wo, A, B, C, bqk, fbias, use_bin, use_beta,
                     use_fbias):
    m = {
        "x": np.ascontiguousarray(xs, dtype=F32),
        "wqk": wqk, "wv": wv, "wo": wo,
        "tabA": np.ascontiguousarray(A[:, :, 0:ROT]).reshape(TOK, 2 * ROT).astype(BF16),
        "tabB": np.ascontiguousarray(B[:, :, 0:ROT]).reshape(TOK, 2 * ROT).astype(BF16),
        "gpass": np.broadcast_to(
            A[0, :, ROT:HD].reshape(1, 2 * 32), (128, 2 * 32)).astype(BF16),
        "ident": np.eye(128, dtype=BF16),
    }
    if use_beta:
        m["tabC"] = C.reshape(TOK, 2 * HD).astype(BF16)
    if use_bin:
        m["bqk"] = bqk
    if use_fbias:
        m["fbias"] = fbias
    return m


def kernel(x, w_in, b_in, w_out, b_out, q_gamma, q_beta, k_gamma, k_beta):
    x = np.asarray(x)
    Bb, T, Hh, Ww, Cc = x.shape
    assert (Bb, Hh, Ww, Cc) == (1, SEQ, SEQ, EMBED) and T == 16

    b_in = np.asarray(b_in)
    b_out = np.asarray(b_out)
    q_beta_n = np.asarray(q_beta)
    k_beta_n = np.asarray(k_beta)
    use_bin = bool(np.any(np.asarray(b_in, F32) != 0.0))
    use_beta = bool(np.any(np.asarray(q_beta_n, F32) != 0.0)
                    or np.any(np.asarray(k_beta_n, F32) != 0.0))
    bv = np.asarray(b_in, F32)[1536:2304]
    fb = bv @ np.asarray(w_out, F32) + np.asarray(b_out, F32)
    use_fbias = bool(np.any(fb != 0.0))

    wqk, wv, wo = _prep_weights(w_in, w_out)
    A, B, C = _tables(q_gamma, q_beta_n, k_gamma, k_beta_n)
    P = _perm96()
    bqk = None
    if use_bin:
        bq = np.asarray(b_in, F32)[0:768].reshape(NHEADS, HD)[:, P]
        bk = np.asarray(b_in, F32)[768:1536].reshape(NHEADS, HD)[:, P]
        bqk = np.ascontiguousarray(
            np.stack([bq, bk], 1).reshape(1, NHEADS * 192), F32)
    fbias = np.broadcast_to(fb.astype(F32), (128, EMBED)).copy() if use_fbias else None

    key = (2, NHEADS, use_bin, use_beta, use_fbias)
    if key not in _CACHE:
        _CACHE[key] = build_program(2, NHEADS, use_bin, use_beta, use_fbias)
    nc = _CACHE[key]

    xf = x.reshape(T, TOK, EMBED).astype(F32)
    in_maps = []
    for c in range(NCORES):
        xs = xf[2 * c:2 * c + 2]
        in_maps.append(_inputs_for_core(xs, wqk, wv, wo, A, B, C, bqk, fbias,
                                        use_bin, use_beta, use_fbias))

    from concourse.bass_utils import run_bass_kernel_spmd

    res = run_bass_kernel_spmd(nc, in_maps, core_ids=list(range(NCORES)))
    global LAST_RESULT
    LAST_RESULT = res
    outs = [r["out"] for r in res.results]
    out = np.concatenate(outs, axis=0)              # [16, TOK, EMBED]
    return out.reshape(1, T, SEQ, SEQ, EMBED).astype(F32)
